# revision 3
# baseline (speedup 1.0000x reference)
"""Grouped-experts GEMM (MoE ragged dot) on 8 TRN2 NeuronCores.

Reference semantics (jax.lax.ragged_dot):
    for each expert e with contiguous token group [start_e, end_e):
        out[start_e:end_e] = input[start_e:end_e] @ weight[e]
    rows beyond sum(tokens_per_expert) are zero.

Sharding: tensor-parallel over out_features. Every core sees all tokens
(identical expert boundaries -> identical SPMD program on all 8 cores)
and computes a disjoint 512-wide slice of the 4096 output columns, so the
"gather" is a host-side concatenate. Matmuls run as float32r (fp32 read,
FP22 multiply, fp32 accumulate) which streams at bf16-class speed on the
PE when the moving free dim is >= 256.
"""

import sys

import numpy as np

sys.path.insert(0, "/opt/trn_rl_repo")

NUM_TOKENS = 8192
IN_FEATURES = 2048
OUT_FEATURES = 4096
GROUPS = 8
N_CORES = 8

P = 128  # partitions / M-tile
KT = IN_FEATURES // P  # 16 K-tiles of 128
N_CORE = OUT_FEATURES // N_CORES  # 512 output cols per core

_BUILD_CACHE: dict = {}


def _build_program(units: tuple[int, ...]):
    """Build the single SPMD Bass program, specialized to the per-expert
    padded M-tile counts `units` (same on every core)."""
    import concourse.bass as bass
    import concourse.mybir as mybir
    import concourse.tile as tile
    from concourse import bacc

    f32 = mybir.dt.float32
    f32r = mybir.dt.float32r
    U = sum(units)

    nc = bacc.Bacc(None, target_bir_lowering=False)
    x_p = nc.declare_dram_parameter("x", [U, P, KT, P], f32r, isOutput=False)
    w_p = nc.declare_dram_parameter("w", [GROUPS, P, KT, N_CORE], f32r, isOutput=False)
    o_p = nc.declare_dram_parameter("out", [U * P, N_CORE], f32, isOutput=True)

    with tile.TileContext(nc) as tc:
        with (
            tc.tile_pool(name="xp", bufs=4) as xpool,
            tc.tile_pool(name="wp", bufs=3) as wpool,
            tc.tile_pool(name="op", bufs=4) as opool,
            tc.tile_pool(name="ps", bufs=4, space="PSUM") as pspool,
        ):
            m = 0
            for e in range(GROUPS):
                if units[e] == 0:
                    continue
                w_t = wpool.tile([P, KT, N_CORE], f32r)
                nc.sync.dma_start(out=w_t[:], in_=w_p[e])
                for _ in range(units[e]):
                    x_t = xpool.tile([P, KT, P], f32r)
                    nc.sync.dma_start(out=x_t[:], in_=x_p[m])
                    ps = pspool.tile([P, N_CORE], f32)
                    for k in range(KT):
                        nc.tensor.matmul(
                            ps[:],
                            x_t[:, k, :],
                            w_t[:, k, :],
                            start=(k == 0),
                            stop=(k == KT - 1),
                        )
                    o_t = opool.tile([P, N_CORE], f32)
                    nc.vector.tensor_copy(o_t[:], ps[:])
                    nc.scalar.dma_start(
                        out=o_p[m * P : (m + 1) * P, :], in_=o_t[:]
                    )
                    m += 1
    nc.compile()
    return nc


def _get_program(units: tuple[int, ...]):
    if units not in _BUILD_CACHE:
        _BUILD_CACHE[units] = _build_program(units)
    return _BUILD_CACHE[units]


def _segments(tokens_per_expert: np.ndarray, total: int):
    """Per-expert (start, size) with ragged_dot clipping semantics."""
    sizes = []
    start = 0
    for e in range(GROUPS):
        s = int(max(0, tokens_per_expert[e]))
        s = min(s, total - start)
        sizes.append(s)
        start += s
    return sizes


def kernel(input, weight, tokens_per_expert, _trace=False, _trace_kwargs=None):
    from concourse.bass_utils import run_bass_kernel_spmd

    x = np.ascontiguousarray(np.asarray(input, dtype=np.float32))
    w = np.ascontiguousarray(np.asarray(weight, dtype=np.float32))
    tpe = np.asarray(tokens_per_expert, dtype=np.int64)
    T, K = x.shape
    G, K2, N = w.shape
    assert (T, K, G, K2, N) == (NUM_TOKENS, IN_FEATURES, GROUPS, IN_FEATURES, OUT_FEATURES)

    sizes = _segments(tpe, T)
    units = tuple(-(-s // P) for s in sizes)  # ceil(s/128)
    U = sum(units)
    out = np.zeros((T, N), dtype=np.float32)
    if U == 0:
        return out

    # Padded token layout: each expert's rows start at a 128-multiple.
    mstarts = np.concatenate([[0], np.cumsum(units)])[:GROUPS]
    Xp = np.zeros((U * P, K), dtype=np.float32)
    start = 0
    for e in range(GROUPS):
        s = sizes[e]
        if s:
            Xp[mstarts[e] * P : mstarts[e] * P + s] = x[start : start + s]
        start += s

    # x_dram[m, p, kt, t] = Xp[m*128 + t, kt*128 + p]  (contiguous per partition)
    x_dram = np.ascontiguousarray(
        Xp.reshape(U, P, KT, P).transpose(0, 3, 2, 1)
    )
    # w_dram[c][e, p, kt, n] = w[e, kt*128 + p, c*512 + n]
    w_drams = [
        np.ascontiguousarray(
            w[:, :, c * N_CORE : (c + 1) * N_CORE]
            .reshape(G, KT, P, N_CORE)
            .transpose(0, 2, 1, 3)
        )
        for c in range(N_CORES)
    ]

    nc = _get_program(units)
    in_maps = [{"x": x_dram, "w": w_drams[c]} for c in range(N_CORES)]
    kw = dict(_trace_kwargs or {})
    res = run_bass_kernel_spmd(
        nc, in_maps, list(range(N_CORES)), trace=_trace, **kw
    )
    full = np.concatenate(
        [res.results[c]["out"] for c in range(N_CORES)], axis=1
    )  # [U*128, 4096]

    start = 0
    for e in range(GROUPS):
        s = sizes[e]
        if s:
            out[start : start + s] = full[mstarts[e] * P : mstarts[e] * P + s]
        start += s
    if _trace:
        return out, res
    return out
